# revision 2
# baseline (speedup 1.0000x reference)
"""Biaffine labeler kernel for 8x Trainium2 NeuronCores.

Full-input contract: kernel(**inputs) takes the unsharded inputs and
returns the full [8, 256, 50] float32 logits.

Sharding: data-parallel over B — core i handles batch i. Weights and the
bilinear tensor W are replicated.

Per-core pipeline (T=256 tokens, D=1024, DL=512, NL=50 labels):
  1. head rows are gathered on the HOST (head_indices is host-visible and
     gather commutes with the row-wise projection), so the device computes
     selT[c] = (Whead^T chunk) @ head_selT directly in transposed layout
     [128 e, 256 t]; bhead added on ACT during the PSUM->SBUF copy.
  2. dep_label = dep @ Wdep + bdep  (PE bf16, k-outer; DVE adds bias)
  3. P_n = sel @ W[n]^T for all 50 labels (PE; W streamed bf16 or
     scaled fp8-e3m4 depending on the label group)
  4. logits[t,n] = sum_d dep_label[t,d] * P_n[t,d] via DVE
     scalar_tensor_tensor accum_out (scale 1/W8_SCALE for e3m4 groups);
     label bias added at the end.

Scheduling: the PE is pre-warmed with dummy matmuls on a memset tile
during the startup DMA window (the HAM clock gate otherwise runs the
first ~3.4us at 1.2GHz), input DMAs are issued in just-in-time order in
fine chunks, and the drains of the first label groups are deferred until
dep_label exists (PSUM holds: 2 labels * 2 banks + 2 proj banks = 6).
"""

import sys

sys.path.insert(0, "/opt/trn_rl_repo")

import numpy as np
import ml_dtypes

B, T, D = 8, 256, 1024
NL, DL = 50, 512

# Per-group (dtype, n_labels). '8' = fp8 e3m4 (W scaled by W8_SCALE), 'b' = bf16.
GROUPS = [
    ("8", 1), ("8", 1), ("b", 2), ("b", 2),
    ("b", 4), ("b", 4), ("b", 4), ("b", 4), ("b", 4), ("b", 4),
    ("b", 4), ("b", 4), ("b", 4), ("b", 4), ("b", 4),
]
assert sum(sz for _, sz in GROUPS) == NL
N_GROUPS = len(GROUPS)
W8_SCALE = 128.0
PREFETCH = 4  # wg groups issued ahead inside the main loop

# PE pre-warm tuning (counts of dummy N=64 matmuls)
N_PREWARM = 34
N_FILL = 4  # dummy matmuls between sel-proj DMA chunk waits

# constpack layout (f32 columns)
C_BDEP = 0          # [128, 512] bdep broadcast
C_BIAS = 512        # [128, 50] label bias broadcast
C_BHEAD = 562       # [128, 4] bhead chunks (col c = bhead[c*128:(c+1)*128])
C_TOT = 566

BF16 = ml_dtypes.bfloat16
E3M4 = ml_dtypes.float8_e3m4

LAST_RESULTS = None
_NC_CACHE = None


def _group_ranges():
    out = []
    n0 = 0
    for _, sz in GROUPS:
        out.append((n0, n0 + sz))
        n0 += sz
    return out


def _build_nc():
    import concourse.bacc as bacc
    import concourse.mybir as mybir
    import concourse.tile as tile

    bf = mybir.dt.bfloat16
    f8 = mybir.dt.float8e3
    f32 = mybir.dt.float32
    Alu = mybir.AluOpType
    Act = mybir.ActivationFunctionType

    nc = bacc.Bacc(None)

    # --- DRAM I/O ---------------------------------------------------------
    # selpack/deppack: 4 chunks of 1536 cols; chunk q holds
    # [w slabs 2q,2q+1 (512 cols each) | xT slabs 2q,2q+1 (256 cols each)]
    selpack = nc.dram_tensor("selpack", [128, 6144], bf, kind="ExternalInput")
    deppack = nc.dram_tensor("deppack", [128, 6144], bf, kind="ExternalInput")
    constpack = nc.dram_tensor("constpack", [128, C_TOT], f32, kind="ExternalInput")
    ranges = _group_ranges()
    wg_dram = []
    for g, (dt8, sz) in enumerate(GROUPS):
        wg_dram.append(
            nc.dram_tensor(
                f"wg{g}", [128, sz * 2048], f8 if dt8 == "8" else bf,
                kind="ExternalInput",
            )
        )
    out = nc.dram_tensor("out", [256, 64], f32, kind="ExternalOutput")

    def w_slab(sb, k):  # [128, 512] w slab k (Whead or Wdep rows k*128..)
        return sb[:, (k // 2) * 1536 + (k % 2) * 512 :][:, :512]

    def xT_slab(sb, k):  # [128, 256] activation^T slab k
        return sb[:, (k // 2) * 1536 + 1024 + (k % 2) * 256 :][:, :256]

    with tile.TileContext(nc) as tc:
        with (
            tc.sbuf_pool(name="cpool", bufs=1) as cpool,
            tc.sbuf_pool(name="persist", bufs=1) as pers,
            tc.sbuf_pool(name="wpool", bufs=PREFETCH + 1) as wpool,
            tc.sbuf_pool(name="spool", bufs=4) as spool,
            tc.psum_pool(name="ps", bufs=8) as ps,
        ):
            # --- input DMAs, just-in-time order ---------------------------
            sp_sb = cpool.tile([128, 6144], bf)
            dp_sb = cpool.tile([128, 6144], bf)
            cp_sb = cpool.tile([128, C_TOT], f32)
            wg_tiles = {}

            def issue_wg(g):
                dt8, sz = GROUPS[g]
                wt = wpool.tile(
                    [128, sz * 2048], f8 if dt8 == "8" else bf,
                    tag="wg", name=f"wg{g}",
                )
                nc.sync.dma_start(wt[:], wg_dram[g][:])
                wg_tiles[g] = wt

            nc.sync.dma_start(sp_sb[:, :1536], selpack[:, :1536])
            nc.sync.dma_start(sp_sb[:, 1536:3072], selpack[:, 1536:3072])
            nc.sync.dma_start(cp_sb[:], constpack[:])
            nc.sync.dma_start(sp_sb[:, 3072:4608], selpack[:, 3072:4608])
            nc.sync.dma_start(sp_sb[:, 4608:], selpack[:, 4608:])
            issue_wg(0)
            issue_wg(1)
            nc.sync.dma_start(dp_sb[:, :1536], deppack[:, :1536])
            nc.sync.dma_start(dp_sb[:, 1536:3072], deppack[:, 1536:3072])
            nc.sync.dma_start(dp_sb[:, 3072:4608], deppack[:, 3072:4608])
            nc.sync.dma_start(dp_sb[:, 4608:], deppack[:, 4608:])
            issue_wg(2)
            issue_wg(3)

            # --- PE pre-warm on a memset tile -----------------------------
            # Dummy matmuls during the startup DMA window keep the HAM
            # activity monitor busy so real matmuls start at 2.4GHz.
            pw = cpool.tile([128, 192], bf)
            nc.vector.memset(pw[:], 0.5)
            pw_ps = ps.tile([128, 512], f32, tag="ps", name="pw")

            def prewarm(n):
                for _ in range(n):
                    nc.tensor.matmul(
                        pw_ps[:, :64], lhsT=pw[:, :128], rhs=pw[:, 128:192],
                        start=True, stop=True,
                    )

            prewarm(N_PREWARM)

            # --- sel projection, transposed: selT[c] = [128 e, 256 t] -----
            psc = [
                ps.tile([128, 512], f32, tag="ps", name=f"psc{c}") for c in range(4)
            ]
            for k in range(8):
                if k in (2, 4, 6):
                    prewarm(N_FILL)
                for c in range(4):
                    nc.tensor.matmul(
                        psc[c][:, :256],
                        lhsT=w_slab(sp_sb, k)[:, c * 128 : (c + 1) * 128],
                        rhs=xT_slab(sp_sb, k),
                        start=(k == 0),
                        stop=(k == 7),
                    )
            selT = []
            for c in range(4):
                sc = pers.tile([128, 256], bf, tag=f"sel{c}", name=f"sel{c}")
                nc.scalar.activation(
                    sc[:],
                    psc[c][:, :256],
                    Act.Identity,
                    bias=cp_sb[:, C_BHEAD + c : C_BHEAD + c + 1],
                    scale=1.0,
                )
                selT.append(sc)

            # --- output accumulators --------------------------------------
            out_sb = []
            for m in range(2):
                om = pers.tile([128, 64], f32, tag=f"out{m}", name=f"out{m}")
                out_sb.append(om)

            dep_label = []
            deferred = []

            def drain_bank(g, n, m, pbt):
                prod = spool.tile(
                    [128, 512], f32, tag="prod", name=f"prod_{g}_{n}_{m}"
                )
                nc.vector.scalar_tensor_tensor(
                    out=prod[:],
                    in0=pbt[:],
                    scalar=(1.0 / W8_SCALE) if GROUPS[g][0] == "8" else 1.0,
                    in1=dep_label[m][:],
                    op0=Alu.mult,
                    op1=Alu.mult,
                    accum_out=out_sb[m][:, n : n + 1],
                )

            def do_group(g, defer_drain=False):
                n0, n1 = ranges[g]
                wg_sb = wg_tiles[g]
                for li, n in enumerate(range(n0, n1)):
                    for m in range(2):
                        pbt = ps.tile(
                            [128, 512], f32, tag="ps", name=f"pb_{g}_{li}_{m}"
                        )
                        for k in range(4):
                            nc.tensor.matmul(
                                pbt[:],
                                lhsT=selT[k][:, m * 128 : (m + 1) * 128],
                                rhs=wg_sb[
                                    :, (li * 4 + k) * 512 : (li * 4 + k + 1) * 512
                                ],
                                start=(k == 0),
                                stop=(k == 3),
                            )
                        if defer_drain:
                            deferred.append((g, n, m, pbt))
                        else:
                            drain_bank(g, n, m, pbt)
                if g + PREFETCH < N_GROUPS:
                    issue_wg(g + PREFETCH)

            # groups 0,1 run before dep_label exists; their drains wait.
            do_group(0, defer_drain=True)
            do_group(1, defer_drain=True)

            # --- dep projection: dep_label[m] = [128 t, 512 d] f32 --------
            pd = [ps.tile([128, 512], f32, tag="ps", name=f"pd{m}") for m in range(2)]
            for k in range(8):
                for m in range(2):
                    nc.tensor.matmul(
                        pd[m][:],
                        lhsT=xT_slab(dp_sb, k)[:, m * 128 : (m + 1) * 128],
                        rhs=w_slab(dp_sb, k),
                        start=(k == 0),
                        stop=(k == 7),
                    )
            for m in range(2):
                dl = pers.tile([128, 512], f32, tag=f"dl{m}", name=f"dl{m}")
                nc.vector.tensor_tensor(
                    dl[:], pd[m][:], cp_sb[:, C_BDEP : C_BDEP + 512], Alu.add
                )
                dep_label.append(dl)

            for (g, n, m, pbt) in deferred:
                drain_bank(g, n, m, pbt)
            deferred.clear()

            for g in range(2, N_GROUPS):
                do_group(g)

            # --- add label bias, single merged store ----------------------
            fin = pers.tile([128, 128], f32, tag="fin", name="fin")
            for m in range(2):
                nc.vector.tensor_tensor(
                    fin[:, m * 64 : m * 64 + NL],
                    out_sb[m][:, :NL],
                    cp_sb[:, C_BIAS : C_BIAS + NL],
                    Alu.add,
                )
            out_v = out.rearrange("(m p) n -> p m n", m=2)
            fin_v = fin.rearrange("p (m n) -> p m n", m=2)
            nc.sync.dma_start(out_v[:, :, :NL], fin_v[:, :, :NL])

    nc.finalize()
    return nc


def _pack_w(Wm):  # [1024, 512] -> [128, 4096] slab-major
    return Wm.reshape(8, 128, 512).transpose(1, 0, 2).reshape(128, 4096)


def _pack_x(x):  # [256, 1024] -> [128, 2048] slab-major (x^T chunks)
    return x.T.reshape(8, 128, 256).transpose(1, 0, 2).reshape(128, 2048)


def _pack_chunks(w_h, xT_h):
    """Interleave [w slab 2q, 2q+1 | xT slab 2q, 2q+1] -> [128, 6144]."""
    cols = []
    for q in range(4):
        cols.append(w_h[:, q * 1024 : (q + 1) * 1024])
        cols.append(xT_h[:, q * 512 : (q + 1) * 512])
    return np.concatenate(cols, axis=1)


def _stage_shared(Wdep, bdep, Whead, bhead, W, bias):
    whead_h = _pack_w(Whead)
    wdep_h = _pack_w(Wdep)

    # W[n, d, e] -> WT[n, k, p, d] = W[n, d, k*128+p]
    WT = np.ascontiguousarray(W.transpose(0, 2, 1)).reshape(NL, 4, 128, 512)
    wg_h = {}
    for g, (n0, n1) in enumerate(_group_ranges()):
        blk = WT[n0:n1]  # [ng, 4, 128, 512]
        ng = n1 - n0
        flat = blk.transpose(2, 0, 1, 3).reshape(128, ng * 2048)
        if GROUPS[g][0] == "8":
            wg_h[f"wg{g}"] = np.asarray(flat * W8_SCALE, dtype=E3M4)
        else:
            wg_h[f"wg{g}"] = np.asarray(flat, dtype=BF16)

    constpack = np.zeros((128, C_TOT), dtype=np.float32)
    constpack[:, C_BDEP : C_BDEP + 512] = bdep[None, :]
    constpack[:, C_BIAS : C_BIAS + NL] = bias[None, :]
    constpack[:, C_BHEAD : C_BHEAD + 4] = bhead.reshape(4, 128).T

    return {
        "whead_h": whead_h,
        "wdep_h": wdep_h,
        "wg_h": wg_h,
        "constpack": constpack,
    }


def _stage_core(shared, dep_b, head_b, idx_b):
    head_sel = head_b[np.asarray(idx_b, dtype=np.int64)]  # host gather [256,1024]
    selpack = np.ascontiguousarray(
        _pack_chunks(shared["whead_h"], _pack_x(head_sel))
    ).astype(BF16)
    deppack = np.ascontiguousarray(
        _pack_chunks(shared["wdep_h"], _pack_x(dep_b))
    ).astype(BF16)
    m = {"selpack": selpack, "deppack": deppack, "constpack": shared["constpack"]}
    m.update(shared["wg_h"])
    return m


def kernel(dep, head, head_indices, mask, Wdep, bdep, Whead, bhead, W, bias):
    global LAST_RESULTS, _NC_CACHE
    from concourse.bass_utils import run_bass_kernel_spmd

    dep = np.asarray(dep, dtype=np.float32)
    head = np.asarray(head, dtype=np.float32)
    head_indices = np.asarray(head_indices)
    Wdep = np.asarray(Wdep, dtype=np.float32)
    bdep = np.asarray(bdep, dtype=np.float32)
    Whead = np.asarray(Whead, dtype=np.float32)
    bhead = np.asarray(bhead, dtype=np.float32)
    W = np.asarray(W, dtype=np.float32)
    bias = np.asarray(bias, dtype=np.float32)

    if _NC_CACHE is None:
        _NC_CACHE = _build_nc()
    nc = _NC_CACHE

    shared = _stage_shared(Wdep, bdep, Whead, bhead, W, bias)
    in_maps = [
        _stage_core(shared, dep[b], head[b], head_indices[b]) for b in range(B)
    ]

    res = run_bass_kernel_spmd(nc, in_maps, list(range(B)))
    LAST_RESULTS = res
    outs = [
        np.asarray(res.results[b]["out"][:, :NL], dtype=np.float32) for b in range(B)
    ]
    return np.stack(outs, axis=0)


# revision 18
# speedup vs baseline: 1.0328x; 1.0328x over previous
"""Biaffine labeler kernel for 8x Trainium2 NeuronCores.

Full-input contract: kernel(**inputs) takes the unsharded inputs and
returns the full [8, 256, 50] float32 logits.

Sharding: data-parallel over B — core i handles batch i. Weights and the
bilinear tensor W are replicated.

Per-core pipeline (T=256 tokens, D=1024, DL=512, NL=50 labels):
  1. head rows are gathered on the HOST (head_indices is host-visible and
     gather commutes with the row-wise projection), so the device computes
     selT[c] = (Whead^T chunk) @ head_selT directly in transposed layout
     [128 e, 256 t]; bhead added on ACT during the PSUM->SBUF copy.
  2. dep_label = dep @ Wdep + bdep  (PE bf16, k-outer; DVE adds bias)
  3. P_n = sel @ W[n]^T for all 50 labels (PE; W streamed bf16 or
     scaled fp8-e3m4 depending on the label group)
  4. logits[t,n] = sum_d dep_label[t,d] * P_n[t,d] via DVE
     scalar_tensor_tensor accum_out (scale 1/W8_SCALE for e3m4 groups);
     label bias added at the end.

Scheduling: the PE is pre-warmed with dummy matmuls on a memset tile
during the startup DMA window (the HAM clock gate otherwise runs the
first ~3.4us at 1.2GHz), input DMAs are issued in just-in-time order in
fine chunks, and the drains of the first label groups are deferred until
dep_label exists (PSUM holds: 2 labels * 2 banks + 2 proj banks = 6).
"""

import sys

sys.path.insert(0, "/opt/trn_rl_repo")

import numpy as np
import ml_dtypes

B, T, D = 8, 256, 1024
NL, DL = 50, 512

# Per-group (dtype, n_labels). '8' = fp8 e3m4 (W scaled by W8_SCALE), 'b' = bf16.
GROUPS = [
    ("8", 1), ("8", 1), ("8", 2), ("8", 2),
    ("8", 4), ("8", 4), ("8", 4), ("8", 4), ("8", 4), ("8", 4),
    ("b", 4), ("b", 4), ("b", 4), ("b", 4), ("b", 4),
]
assert sum(sz for _, sz in GROUPS) == NL
N_GROUPS = len(GROUPS)
W8_SCALE = 128.0       # e3m4 W scale ('8' groups)
WD_SCALE = 1024.0      # e4m3 W scale ('d' groups, DoubleRow)
SELD_SCALE = 16.0      # e4m3 sel scale ('d' groups)
PREFETCH = 4  # wg groups issued ahead inside the main loop

# PE pre-warm tuning (count of dummy N=64 matmuls; the tile scheduler
# hoists them ahead of the DMA-gated real matmuls, so they must cover
# >=3.4us of continuous PE busy to release the HAM clock throttle AND
# end roughly when the first selpack half lands)
N_PREWARM = 60

# constpack layout (f32 columns)
C_BDEP = 0          # [128, 512] bdep broadcast
C_BIAS = 512        # [128, 50] label bias broadcast
C_BHEAD = 562       # [128, 4] bhead chunks (col c = bhead[c*128:(c+1)*128])
C_BHEAD16 = 566     # [128, 4] bhead chunks * SELD_SCALE (for 'd' sel8 copies)
C_TOT = 570

BF16 = ml_dtypes.bfloat16
E3M4 = ml_dtypes.float8_e3m4

LAST_RESULTS = None
_NC_CACHE = None


def _group_ranges():
    out = []
    n0 = 0
    for _, sz in GROUPS:
        out.append((n0, n0 + sz))
        n0 += sz
    return out


def _build_nc():
    import concourse.bacc as bacc
    import concourse.mybir as mybir
    import concourse.tile as tile

    bf = mybir.dt.bfloat16
    f8 = mybir.dt.float8e3
    f8d = mybir.dt.float8e4
    f32 = mybir.dt.float32
    Alu = mybir.AluOpType
    Act = mybir.ActivationFunctionType
    DR = mybir.MatmulPerfMode.DoubleRow

    def g_dt(dt8):
        return {"8": f8, "d": f8d}.get(dt8, bf)

    nc = bacc.Bacc(None)

    # --- DRAM I/O ---------------------------------------------------------
    # selpack/deppack: 4 chunks of 1536 cols; chunk q holds
    # [w slabs 2q,2q+1 (512 cols each) | xT slabs 2q,2q+1 (256 cols each)]
    selpack = nc.dram_tensor("selpack", [128, 6144], bf, kind="ExternalInput")
    deppack = nc.dram_tensor("deppack", [128, 6144], bf, kind="ExternalInput")
    constpack = nc.dram_tensor("constpack", [128, C_TOT], f32, kind="ExternalInput")
    ranges = _group_ranges()
    wg_dram = []
    for g, (dt8, sz) in enumerate(GROUPS):
        wg_dram.append(
            nc.dram_tensor(
                f"wg{g}", [128, sz * 2048], g_dt(dt8), kind="ExternalInput"
            )
        )
    out = nc.dram_tensor("out", [256, 64], f32, kind="ExternalOutput")

    def w_slab(sb, k):  # [128, 512] w slab k (Whead or Wdep rows k*128..)
        return sb[:, (k // 2) * 1536 + (k % 2) * 512 :][:, :512]

    def xT_slab(sb, k):  # [128, 256] activation^T slab k
        return sb[:, (k // 2) * 1536 + 1024 + (k % 2) * 256 :][:, :256]

    with tile.TileContext(nc) as tc:
        with (
            tc.sbuf_pool(name="cpool", bufs=1) as cpool,
            tc.sbuf_pool(name="persist", bufs=1) as pers,
            tc.sbuf_pool(name="wpool", bufs=PREFETCH + 1) as wpool,
            tc.sbuf_pool(name="spool", bufs=4) as spool,
            tc.psum_pool(name="ps", bufs=8) as ps,
        ):
            # --- input DMAs, just-in-time order ---------------------------
            sp_sb = cpool.tile([128, 6144], bf)
            dp_sb = cpool.tile([128, 6144], bf)
            cp_sb = cpool.tile([128, C_TOT], f32)
            wg_tiles = {}

            def issue_wg(g):
                dt8, sz = GROUPS[g]
                wt = wpool.tile(
                    [128, sz * 2048], g_dt(dt8), tag="wg", name=f"wg{g}"
                )
                nc.sync.dma_start(wt[:], wg_dram[g][:])
                wg_tiles[g] = wt

            # big transfers only: small dma_starts run at ~half the ring
            # bandwidth (128 row-descriptors need >=6KB payload each)
            nc.sync.dma_start(sp_sb[:, :3072], selpack[:, :3072])
            nc.sync.dma_start(sp_sb[:, 3072:], selpack[:, 3072:])
            nc.sync.dma_start(cp_sb[:], constpack[:])
            issue_wg(0)
            issue_wg(1)
            nc.sync.dma_start(dp_sb[:, :3072], deppack[:, :3072])
            nc.sync.dma_start(dp_sb[:, 3072:], deppack[:, 3072:])
            issue_wg(2)
            issue_wg(3)

            # --- PE pre-warm on a memset tile -----------------------------
            # Dummy matmuls during the startup DMA window keep the HAM
            # activity monitor busy so real matmuls start at 2.4GHz.
            pw = cpool.tile([128, 192], bf)
            nc.vector.memset(pw[:], 0.5)
            pw_ps = ps.tile([128, 512], f32, tag="ps", name="pw")

            def prewarm(n):
                for _ in range(n):
                    nc.tensor.matmul(
                        pw_ps[:, :64], lhsT=pw[:, :128], rhs=pw[:, 128:192],
                        start=True, stop=True,
                    )

            prewarm(N_PREWARM)

            # --- sel projection, transposed: selT[c] = [128 e, 256 t] -----
            psc = [
                ps.tile([128, 512], f32, tag="ps", name=f"psc{c}") for c in range(4)
            ]
            for k in range(8):
                for c in range(4):
                    nc.tensor.matmul(
                        psc[c][:, :256],
                        lhsT=w_slab(sp_sb, k)[:, c * 128 : (c + 1) * 128],
                        rhs=xT_slab(sp_sb, k),
                        start=(k == 0),
                        stop=(k == 7),
                    )
            selT = []
            for c in range(4):
                sc = pers.tile([128, 256], bf, tag=f"sel{c}", name=f"sel{c}")
                nc.scalar.activation(
                    sc[:],
                    psc[c][:, :256],
                    Act.Identity,
                    bias=cp_sb[:, C_BHEAD + c : C_BHEAD + c + 1],
                    scale=1.0,
                )
                selT.append(sc)

            # e4m3 paired-sel copies for DoubleRow groups:
            # sel8[j] cols = sub*256 + t, sub -> e-chunk 2j+sub, scaled 16x
            sel8 = []
            if any(dt8 == "d" for dt8, _ in GROUPS):
                for j in range(2):
                    s8 = pers.tile([128, 512], f8d, tag=f"sel8_{j}", name=f"sel8_{j}")
                    for sub in range(2):
                        nc.scalar.activation(
                            s8[:, sub * 256 : (sub + 1) * 256],
                            psc[2 * j + sub][:, :256],
                            Act.Identity,
                            bias=cp_sb[:, C_BHEAD16 + 2 * j + sub : C_BHEAD16 + 2 * j + sub + 1],
                            scale=SELD_SCALE,
                        )
                    sel8.append(s8)

            # --- output accumulators --------------------------------------
            out_sb = []
            for m in range(2):
                om = pers.tile([128, 64], f32, tag=f"out{m}", name=f"out{m}")
                out_sb.append(om)

            dep_label = []
            deferred = []

            DRAIN_SCALE = {
                "8": 1.0 / W8_SCALE,
                "d": 1.0 / (WD_SCALE * SELD_SCALE),
                "b": 1.0,
            }

            def drain_bank(g, n, m, pbt):
                prod = spool.tile(
                    [128, 512], f32, tag="prod", name=f"prod_{g}_{n}_{m}"
                )
                nc.vector.scalar_tensor_tensor(
                    out=prod[:],
                    in0=pbt[:],
                    scalar=DRAIN_SCALE[GROUPS[g][0]],
                    in1=dep_label[m][:],
                    op0=Alu.mult,
                    op1=Alu.mult,
                    accum_out=out_sb[m][:, n : n + 1],
                )

            def do_group(g, defer_drain=False):
                dt8, _ = GROUPS[g]
                n0, n1 = ranges[g]
                wg_sb = wg_tiles[g]
                wg_3d = wg_sb.rearrange("p (k d) -> p k d", d=512)
                for li, n in enumerate(range(n0, n1)):
                    for m in range(2):
                        pbt = ps.tile(
                            [128, 512], f32, tag="ps", name=f"pb_{g}_{li}_{m}"
                        )
                        if dt8 == "d":
                            for j in range(2):
                                s8v = sel8[j].rearrange("p (s t) -> p s t", s=2)
                                nc.tensor.matmul(
                                    pbt[:],
                                    lhsT=s8v[:, :, m * 128 : (m + 1) * 128],
                                    rhs=wg_3d[:, li * 4 + 2 * j : li * 4 + 2 * j + 2, :],
                                    start=(j == 0),
                                    stop=(j == 1),
                                    perf_mode=DR,
                                )
                        else:
                            for k in range(4):
                                nc.tensor.matmul(
                                    pbt[:],
                                    lhsT=selT[k][:, m * 128 : (m + 1) * 128],
                                    rhs=wg_sb[
                                        :, (li * 4 + k) * 512 : (li * 4 + k + 1) * 512
                                    ],
                                    start=(k == 0),
                                    stop=(k == 3),
                                )
                        if defer_drain:
                            deferred.append((g, n, m, pbt))
                        else:
                            drain_bank(g, n, m, pbt)
                if g + PREFETCH < N_GROUPS:
                    issue_wg(g + PREFETCH)

            # groups 0,1 run before dep_label exists; their drains wait.
            do_group(0, defer_drain=True)
            do_group(1, defer_drain=True)

            # --- dep projection: dep_label[m] = [128 t, 512 d] f32 --------
            pd = [ps.tile([128, 512], f32, tag="ps", name=f"pd{m}") for m in range(2)]
            for k in range(8):
                for m in range(2):
                    nc.tensor.matmul(
                        pd[m][:],
                        lhsT=xT_slab(dp_sb, k)[:, m * 128 : (m + 1) * 128],
                        rhs=w_slab(dp_sb, k),
                        start=(k == 0),
                        stop=(k == 7),
                    )
            for m in range(2):
                dl = pers.tile([128, 512], f32, tag=f"dl{m}", name=f"dl{m}")
                nc.vector.tensor_tensor(
                    dl[:], pd[m][:], cp_sb[:, C_BDEP : C_BDEP + 512], Alu.add
                )
                dep_label.append(dl)

            for (g, n, m, pbt) in deferred:
                drain_bank(g, n, m, pbt)
            deferred.clear()

            for g in range(2, N_GROUPS):
                do_group(g)

            # --- add label bias, single merged store ----------------------
            fin = pers.tile([128, 128], f32, tag="fin", name="fin")
            for m in range(2):
                nc.vector.tensor_tensor(
                    fin[:, m * 64 : m * 64 + NL],
                    out_sb[m][:, :NL],
                    cp_sb[:, C_BIAS : C_BIAS + NL],
                    Alu.add,
                )
            out_v = out.rearrange("(m p) n -> p m n", m=2)
            fin_v = fin.rearrange("p (m n) -> p m n", m=2)
            nc.sync.dma_start(out_v[:, :, :NL], fin_v[:, :, :NL])

    nc.finalize()
    return nc


def _pack_w(Wm):  # [1024, 512] -> [128, 4096] slab-major
    return Wm.reshape(8, 128, 512).transpose(1, 0, 2).reshape(128, 4096)


def _pack_x(x):  # [256, 1024] -> [128, 2048] slab-major (x^T chunks)
    return x.T.reshape(8, 128, 256).transpose(1, 0, 2).reshape(128, 2048)


def _pack_chunks(w_h, xT_h):
    """Interleave [w slab 2q, 2q+1 | xT slab 2q, 2q+1] -> [128, 6144]."""
    cols = []
    for q in range(4):
        cols.append(w_h[:, q * 1024 : (q + 1) * 1024])
        cols.append(xT_h[:, q * 512 : (q + 1) * 512])
    return np.concatenate(cols, axis=1)


def _stage_shared(Wdep, bdep, Whead, bhead, W, bias):
    whead_h = _pack_w(Whead)
    wdep_h = _pack_w(Wdep)

    # W[n, d, e] -> WT[n, k, p, d] = W[n, d, k*128+p]
    WT = np.ascontiguousarray(W.transpose(0, 2, 1)).reshape(NL, 4, 128, 512)
    wg_h = {}
    for g, (n0, n1) in enumerate(_group_ranges()):
        blk = WT[n0:n1]  # [ng, 4, 128, 512]
        ng = n1 - n0
        flat = blk.transpose(2, 0, 1, 3).reshape(128, ng * 2048)
        if GROUPS[g][0] == "8":
            wg_h[f"wg{g}"] = np.asarray(flat * W8_SCALE, dtype=E3M4)
        elif GROUPS[g][0] == "d":
            wg_h[f"wg{g}"] = np.asarray(
                np.clip(flat * WD_SCALE, -240.0, 240.0),
                dtype=ml_dtypes.float8_e4m3,
            )
        else:
            wg_h[f"wg{g}"] = np.asarray(flat, dtype=BF16)

    constpack = np.zeros((128, C_TOT), dtype=np.float32)
    constpack[:, C_BDEP : C_BDEP + 512] = bdep[None, :]
    constpack[:, C_BIAS : C_BIAS + NL] = bias[None, :]
    constpack[:, C_BHEAD : C_BHEAD + 4] = bhead.reshape(4, 128).T
    constpack[:, C_BHEAD16 : C_BHEAD16 + 4] = SELD_SCALE * bhead.reshape(4, 128).T

    return {
        "whead_h": whead_h,
        "wdep_h": wdep_h,
        "wg_h": wg_h,
        "constpack": constpack,
    }


def _stage_core(shared, dep_b, head_b, idx_b):
    head_sel = head_b[np.asarray(idx_b, dtype=np.int64)]  # host gather [256,1024]
    selpack = np.ascontiguousarray(
        _pack_chunks(shared["whead_h"], _pack_x(head_sel))
    ).astype(BF16)
    deppack = np.ascontiguousarray(
        _pack_chunks(shared["wdep_h"], _pack_x(dep_b))
    ).astype(BF16)
    m = {"selpack": selpack, "deppack": deppack, "constpack": shared["constpack"]}
    m.update(shared["wg_h"])
    return m


def kernel(dep, head, head_indices, mask, Wdep, bdep, Whead, bhead, W, bias):
    global LAST_RESULTS, _NC_CACHE
    from concourse.bass_utils import run_bass_kernel_spmd

    dep = np.asarray(dep, dtype=np.float32)
    head = np.asarray(head, dtype=np.float32)
    head_indices = np.asarray(head_indices)
    Wdep = np.asarray(Wdep, dtype=np.float32)
    bdep = np.asarray(bdep, dtype=np.float32)
    Whead = np.asarray(Whead, dtype=np.float32)
    bhead = np.asarray(bhead, dtype=np.float32)
    W = np.asarray(W, dtype=np.float32)
    bias = np.asarray(bias, dtype=np.float32)

    if _NC_CACHE is None:
        _NC_CACHE = _build_nc()
    nc = _NC_CACHE

    shared = _stage_shared(Wdep, bdep, Whead, bhead, W, bias)
    in_maps = [
        _stage_core(shared, dep[b], head[b], head_indices[b]) for b in range(B)
    ]

    res = run_bass_kernel_spmd(nc, in_maps, list(range(B)))
    LAST_RESULTS = res
    outs = [
        np.asarray(res.results[b]["out"][:, :NL], dtype=np.float32) for b in range(B)
    ]
    return np.stack(outs, axis=0)


# revision 22
# speedup vs baseline: 1.0439x; 1.0107x over previous
"""Biaffine labeler kernel for 8x Trainium2 NeuronCores.

Full-input contract: kernel(**inputs) takes the unsharded inputs and
returns the full [8, 256, 50] float32 logits.

Sharding: data-parallel over B — core i handles batch i. Weights and the
bilinear tensor W are replicated.

Per-core pipeline (T=256 tokens, D=1024, DL=512, NL=50 labels):
  1. head rows are gathered on the HOST (head_indices is host-visible and
     gather commutes with the row-wise projection), so the device computes
     selT[c] = (Whead^T chunk) @ head_selT directly in transposed layout
     [128 e, 256 t]; bhead added on ACT during the PSUM->SBUF copy.
  2. dep_label = dep @ Wdep + bdep  (PE bf16, k-outer; DVE adds bias)
  3. P_n = sel @ W[n]^T for all 50 labels (PE; W streamed bf16 or
     scaled fp8-e3m4 depending on the label group)
  4. logits[t,n] = sum_d dep_label[t,d] * P_n[t,d] via DVE
     scalar_tensor_tensor accum_out (scale 1/W8_SCALE for e3m4 groups);
     label bias added at the end.

Scheduling: the PE is pre-warmed with dummy matmuls on a memset tile
during the startup DMA window (the HAM clock gate otherwise runs the
first ~3.4us at 1.2GHz), input DMAs are issued in just-in-time order in
fine chunks, and the drains of the first label groups are deferred until
dep_label exists (PSUM holds: 2 labels * 2 banks + 2 proj banks = 6).
"""

import sys

sys.path.insert(0, "/opt/trn_rl_repo")

import numpy as np
import ml_dtypes

B, T, D = 8, 256, 1024
NL, DL = 50, 512

# Per-group (dtype, n_labels). '8' = fp8 e3m4 (W scaled by W8_SCALE), 'b' = bf16.
GROUPS = [
    ("8", 1), ("8", 1), ("8", 2), ("8", 2),
    ("8", 4), ("8", 4), ("8", 4), ("8", 4), ("8", 4), ("8", 4),
    ("b", 4), ("b", 4), ("b", 4), ("b", 4), ("b", 4),
]
assert sum(sz for _, sz in GROUPS) == NL
N_GROUPS = len(GROUPS)
W8_SCALE = 128.0       # e3m4 W scale ('8' groups)
WD_SCALE = 1024.0      # e4m3 W scale ('d' groups, DoubleRow)
SELD_SCALE = 16.0      # e4m3 sel scale ('d' groups)
PREFETCH = 4  # wg groups issued ahead inside the main loop

# PE pre-warm tuning (count of dummy N=64 matmuls; the tile scheduler
# hoists them ahead of the DMA-gated real matmuls, so they must cover
# >=3.4us of continuous PE busy to release the HAM clock throttle AND
# end roughly when the first selpack half lands)
N_PREWARM = 60

# constpack layout (f32 columns)
C_BDEP = 0          # [128, 512] bdep broadcast
C_BIAS = 512        # [128, 50] label bias broadcast
C_BHEAD = 562       # [128, 4] bhead chunks (col c = bhead[c*128:(c+1)*128])
C_BHEAD16 = 566     # [128, 4] bhead chunks * SELD_SCALE (for 'd' sel8 copies)
C_TOT = 570

BF16 = ml_dtypes.bfloat16
E3M4 = ml_dtypes.float8_e3m4

LAST_RESULTS = None
_NC_CACHE = None


def _group_ranges():
    out = []
    n0 = 0
    for _, sz in GROUPS:
        out.append((n0, n0 + sz))
        n0 += sz
    return out


def _build_nc():
    import concourse.bacc as bacc
    import concourse.mybir as mybir
    import concourse.tile as tile

    bf = mybir.dt.bfloat16
    f8 = mybir.dt.float8e3
    f8d = mybir.dt.float8e4
    f32 = mybir.dt.float32
    Alu = mybir.AluOpType
    Act = mybir.ActivationFunctionType
    DR = mybir.MatmulPerfMode.DoubleRow

    def g_dt(dt8):
        return {"8": f8, "d": f8d}.get(dt8, bf)

    nc = bacc.Bacc(None)

    # --- DRAM I/O ---------------------------------------------------------
    # selpack/deppack: 4 chunks of 1536 cols; chunk q holds
    # [w slabs 2q,2q+1 (512 cols each) | xT slabs 2q,2q+1 (256 cols each)]
    selpack = nc.dram_tensor("selpack", [128, 6144], bf, kind="ExternalInput")
    deppack = nc.dram_tensor("deppack", [128, 6144], bf, kind="ExternalInput")
    constpack = nc.dram_tensor("constpack", [128, C_TOT], f32, kind="ExternalInput")
    ranges = _group_ranges()
    wg_dram = []
    for g, (dt8, sz) in enumerate(GROUPS):
        wg_dram.append(
            nc.dram_tensor(
                f"wg{g}", [128, sz * 2048], g_dt(dt8), kind="ExternalInput"
            )
        )
    out = nc.dram_tensor("out", [256, 64], f32, kind="ExternalOutput")

    def w_slab(sb, k):  # [128, 512] w slab k (Whead or Wdep rows k*128..)
        return sb[:, (k // 2) * 1536 + (k % 2) * 512 :][:, :512]

    def xT_slab(sb, k):  # [128, 256] activation^T slab k
        return sb[:, (k // 2) * 1536 + 1024 + (k % 2) * 256 :][:, :256]

    with tile.TileContext(nc) as tc:
        with (
            tc.sbuf_pool(name="cpool", bufs=1) as cpool,
            tc.sbuf_pool(name="persist", bufs=1) as pers,
            tc.sbuf_pool(name="wpool", bufs=PREFETCH + 1) as wpool,
            tc.sbuf_pool(name="spool", bufs=4) as spool,
            tc.psum_pool(name="ps", bufs=8) as ps,
        ):
            # --- input DMAs, just-in-time order ---------------------------
            sp_sb = cpool.tile([128, 6144], bf)
            dp_sb = cpool.tile([128, 6144], bf)
            cp_sb = cpool.tile([128, C_TOT], f32)
            wg_tiles = {}

            def issue_wg(g):
                dt8, sz = GROUPS[g]
                wt = wpool.tile(
                    [128, sz * 2048], g_dt(dt8), tag="wg", name=f"wg{g}"
                )
                nc.sync.dma_start(wt[:], wg_dram[g][:])
                wg_tiles[g] = wt

            # big transfers only: small dma_starts run at ~half the ring
            # bandwidth (128 row-descriptors need >=6KB payload each)
            nc.sync.dma_start(sp_sb[:, :3072], selpack[:, :3072])
            nc.sync.dma_start(sp_sb[:, 3072:], selpack[:, 3072:])
            nc.sync.dma_start(cp_sb[:], constpack[:])
            issue_wg(0)
            issue_wg(1)
            nc.sync.dma_start(dp_sb[:, :3072], deppack[:, :3072])
            nc.sync.dma_start(dp_sb[:, 3072:], deppack[:, 3072:])
            issue_wg(2)
            issue_wg(3)

            # --- PE pre-warm on a memset tile -----------------------------
            # Dummy matmuls during the startup DMA window keep the HAM
            # activity monitor busy so real matmuls start at 2.4GHz.
            pw = cpool.tile([128, 192], bf)
            nc.vector.memset(pw[:], 0.5)
            pw_ps = ps.tile([128, 512], f32, tag="ps", name="pw")

            def prewarm(n):
                for _ in range(n):
                    nc.tensor.matmul(
                        pw_ps[:, :64], lhsT=pw[:, :128], rhs=pw[:, 128:192],
                        start=True, stop=True,
                    )

            prewarm(N_PREWARM)

            # --- sel projection, transposed: selT[c] = [128 e, 256 t] -----
            psc = [
                ps.tile([128, 512], f32, tag="ps", name=f"psc{c}") for c in range(4)
            ]
            for k in range(8):
                for c in range(4):
                    nc.tensor.matmul(
                        psc[c][:, :256],
                        lhsT=w_slab(sp_sb, k)[:, c * 128 : (c + 1) * 128],
                        rhs=xT_slab(sp_sb, k),
                        start=(k == 0),
                        stop=(k == 7),
                    )
            selT = []
            for c in range(4):
                sc = pers.tile([128, 256], bf, tag=f"sel{c}", name=f"sel{c}")
                nc.scalar.activation(
                    sc[:],
                    psc[c][:, :256],
                    Act.Identity,
                    bias=cp_sb[:, C_BHEAD + c : C_BHEAD + c + 1],
                    scale=1.0,
                )
                selT.append(sc)

            # e4m3 paired-sel copies for DoubleRow groups:
            # sel8[j] cols = sub*256 + t, sub -> e-chunk 2j+sub, scaled 16x
            sel8 = []
            if any(dt8 == "d" for dt8, _ in GROUPS):
                for j in range(2):
                    s8 = pers.tile([128, 512], f8d, tag=f"sel8_{j}", name=f"sel8_{j}")
                    for sub in range(2):
                        nc.scalar.activation(
                            s8[:, sub * 256 : (sub + 1) * 256],
                            psc[2 * j + sub][:, :256],
                            Act.Identity,
                            bias=cp_sb[:, C_BHEAD16 + 2 * j + sub : C_BHEAD16 + 2 * j + sub + 1],
                            scale=SELD_SCALE,
                        )
                    sel8.append(s8)

            # --- output accumulators --------------------------------------
            out_sb = []
            for m in range(2):
                om = pers.tile([128, 64], f32, tag=f"out{m}", name=f"out{m}")
                out_sb.append(om)

            dep_label = []
            deferred = []

            DRAIN_SCALE = {
                "8": 1.0 / W8_SCALE,
                "d": 1.0 / (WD_SCALE * SELD_SCALE),
                "b": 1.0,
            }

            def drain_bank(g, n, m, pbt):
                prod = spool.tile(
                    [128, 512], f32, tag="prod", name=f"prod_{g}_{n}_{m}"
                )
                nc.vector.scalar_tensor_tensor(
                    out=prod[:],
                    in0=pbt[:],
                    scalar=DRAIN_SCALE[GROUPS[g][0]],
                    in1=dep_label[m][:],
                    op0=Alu.mult,
                    op1=Alu.mult,
                    accum_out=out_sb[m][:, n : n + 1],
                )

            def do_group(g, defer_drain=False):
                dt8, _ = GROUPS[g]
                n0, n1 = ranges[g]
                wg_sb = wg_tiles[g]
                wg_3d = wg_sb.rearrange("p (k d) -> p k d", d=512)
                for li, n in enumerate(range(n0, n1)):
                    for m in range(2):
                        pbt = ps.tile(
                            [128, 512], f32, tag="ps", name=f"pb_{g}_{li}_{m}"
                        )
                        if dt8 == "d":
                            for j in range(2):
                                s8v = sel8[j].rearrange("p (s t) -> p s t", s=2)
                                nc.tensor.matmul(
                                    pbt[:],
                                    lhsT=s8v[:, :, m * 128 : (m + 1) * 128],
                                    rhs=wg_3d[:, li * 4 + 2 * j : li * 4 + 2 * j + 2, :],
                                    start=(j == 0),
                                    stop=(j == 1),
                                    perf_mode=DR,
                                )
                        else:
                            for k in range(4):
                                nc.tensor.matmul(
                                    pbt[:],
                                    lhsT=selT[k][:, m * 128 : (m + 1) * 128],
                                    rhs=wg_sb[
                                        :, (li * 4 + k) * 512 : (li * 4 + k + 1) * 512
                                    ],
                                    start=(k == 0),
                                    stop=(k == 3),
                                )
                        if defer_drain:
                            deferred.append((g, n, m, pbt))
                        else:
                            drain_bank(g, n, m, pbt)
                if g + PREFETCH < N_GROUPS:
                    issue_wg(g + PREFETCH)

            # groups 0,1 run before dep_label exists; their drains wait.
            do_group(0, defer_drain=True)
            do_group(1, defer_drain=True)

            # --- dep projection: dep_label[m] = [128 t, 512 d] f32 --------
            pd = [ps.tile([128, 512], f32, tag="ps", name=f"pd{m}") for m in range(2)]
            for k in range(8):
                for m in range(2):
                    nc.tensor.matmul(
                        pd[m][:],
                        lhsT=xT_slab(dp_sb, k)[:, m * 128 : (m + 1) * 128],
                        rhs=w_slab(dp_sb, k),
                        start=(k == 0),
                        stop=(k == 7),
                    )
            for m in range(2):
                dl = pers.tile([128, 512], f32, tag=f"dl{m}", name=f"dl{m}")
                nc.vector.tensor_tensor(
                    dl[:], pd[m][:], cp_sb[:, C_BDEP : C_BDEP + 512], Alu.add
                )
                dep_label.append(dl)

            for (g, n, m, pbt) in deferred:
                drain_bank(g, n, m, pbt)
            deferred.clear()

            for g in range(2, N_GROUPS):
                do_group(g)

            # --- add label bias, single merged store ----------------------
            fin = pers.tile([128, 128], f32, tag="fin", name="fin")
            for m in range(2):
                nc.vector.tensor_tensor(
                    fin[:, m * 64 : m * 64 + NL],
                    out_sb[m][:, :NL],
                    cp_sb[:, C_BIAS : C_BIAS + NL],
                    Alu.add,
                )
            out_v = out.rearrange("(m p) n -> p m n", m=2)
            fin_v = fin.rearrange("p (m n) -> p m n", m=2)
            nc.sync.dma_start(out_v[:, :, :NL], fin_v[:, :, :NL])

    nc.finalize()
    return nc


def _pack_w(Wm):  # [1024, 512] -> [128, 4096] slab-major
    return Wm.reshape(8, 128, 512).transpose(1, 0, 2).reshape(128, 4096)


def _pack_x(x):  # [256, 1024] -> [128, 2048] slab-major (x^T chunks)
    return x.T.reshape(8, 128, 256).transpose(1, 0, 2).reshape(128, 2048)


def _pack_chunks(w_h, xT_h):
    """Interleave [w slab 2q, 2q+1 | xT slab 2q, 2q+1] -> [128, 6144]."""
    cols = []
    for q in range(4):
        cols.append(w_h[:, q * 1024 : (q + 1) * 1024])
        cols.append(xT_h[:, q * 512 : (q + 1) * 512])
    return np.concatenate(cols, axis=1)


def _stage_shared(Wdep, bdep, Whead, bhead, W, bias):
    whead_h = _pack_w(Whead)
    wdep_h = _pack_w(Wdep)

    # W[n, d, e] -> WT[n, k, p, d] = W[n, d, k*128+p]
    WT = np.ascontiguousarray(W.transpose(0, 2, 1)).reshape(NL, 4, 128, 512)
    wg_h = {}
    for g, (n0, n1) in enumerate(_group_ranges()):
        blk = WT[n0:n1]  # [ng, 4, 128, 512]
        ng = n1 - n0
        flat = blk.transpose(2, 0, 1, 3).reshape(128, ng * 2048)
        if GROUPS[g][0] == "8":
            wg_h[f"wg{g}"] = np.asarray(flat * W8_SCALE, dtype=E3M4)
        elif GROUPS[g][0] == "d":
            wg_h[f"wg{g}"] = np.asarray(
                np.clip(flat * WD_SCALE, -240.0, 240.0),
                dtype=ml_dtypes.float8_e4m3,
            )
        else:
            wg_h[f"wg{g}"] = np.asarray(flat, dtype=BF16)

    constpack = np.zeros((128, C_TOT), dtype=np.float32)
    constpack[:, C_BDEP : C_BDEP + 512] = bdep[None, :]
    constpack[:, C_BIAS : C_BIAS + NL] = bias[None, :]
    constpack[:, C_BHEAD : C_BHEAD + 4] = bhead.reshape(4, 128).T
    constpack[:, C_BHEAD16 : C_BHEAD16 + 4] = SELD_SCALE * bhead.reshape(4, 128).T

    return {
        "whead_h": whead_h,
        "wdep_h": wdep_h,
        "wg_h": wg_h,
        "constpack": constpack,
    }


def _stage_core(shared, dep_b, head_b, idx_b):
    head_sel = head_b[np.asarray(idx_b, dtype=np.int64)]  # host gather [256,1024]
    selpack = np.ascontiguousarray(
        _pack_chunks(shared["whead_h"], _pack_x(head_sel))
    ).astype(BF16)
    deppack = np.ascontiguousarray(
        _pack_chunks(shared["wdep_h"], _pack_x(dep_b))
    ).astype(BF16)
    m = {"selpack": selpack, "deppack": deppack, "constpack": shared["constpack"]}
    m.update(shared["wg_h"])
    return m


def kernel(dep, head, head_indices, mask, Wdep, bdep, Whead, bhead, W, bias):
    global LAST_RESULTS, _NC_CACHE
    from concourse.bass_utils import run_bass_kernel_spmd

    dep = np.asarray(dep, dtype=np.float32)
    head = np.asarray(head, dtype=np.float32)
    head_indices = np.asarray(head_indices)
    Wdep = np.asarray(Wdep, dtype=np.float32)
    bdep = np.asarray(bdep, dtype=np.float32)
    Whead = np.asarray(Whead, dtype=np.float32)
    bhead = np.asarray(bhead, dtype=np.float32)
    W = np.asarray(W, dtype=np.float32)
    bias = np.asarray(bias, dtype=np.float32)

    if _NC_CACHE is None:
        _NC_CACHE = _build_nc()
    nc = _NC_CACHE

    shared = _stage_shared(Wdep, bdep, Whead, bhead, W, bias)
    in_maps = [
        _stage_core(shared, dep[b], head[b], head_indices[b]) for b in range(B)
    ]

    res = run_bass_kernel_spmd(nc, in_maps, list(range(B)))
    LAST_RESULTS = res
    outs = [
        np.asarray(res.results[b]["out"][:, :NL], dtype=np.float32) for b in range(B)
    ]
    return np.stack(outs, axis=0)


# revision 23
# speedup vs baseline: 1.0501x; 1.0059x over previous
"""Biaffine labeler kernel for 8x Trainium2 NeuronCores.

Full-input contract: kernel(**inputs) takes the unsharded inputs and
returns the full [8, 256, 50] float32 logits.

Sharding: data-parallel over B — core i handles batch i. Weights and the
bilinear tensor W are replicated.

Per-core pipeline (T=256 tokens, D=1024, DL=512, NL=50 labels):
  1. head rows are gathered on the HOST (head_indices is host-visible and
     gather commutes with the row-wise projection), so the device computes
     selT[c] = (Whead^T chunk) @ head_selT directly in transposed layout
     [128 e, 256 t]; bhead added on ACT during the PSUM->SBUF copy.
  2. dep_label = dep @ Wdep + bdep  (PE bf16, k-outer; DVE adds bias)
  3. P_n = sel @ W[n]^T for all 50 labels (PE; W streamed bf16 or
     scaled fp8-e3m4 depending on the label group)
  4. logits[t,n] = sum_d dep_label[t,d] * P_n[t,d] via DVE
     scalar_tensor_tensor accum_out (scale 1/W8_SCALE for e3m4 groups);
     label bias added at the end.

Scheduling: the PE is pre-warmed with dummy matmuls on a memset tile
during the startup DMA window (the HAM clock gate otherwise runs the
first ~3.4us at 1.2GHz), input DMAs are issued in just-in-time order in
fine chunks, and the drains of the first label groups are deferred until
dep_label exists (PSUM holds: 2 labels * 2 banks + 2 proj banks = 6).
"""

import sys

sys.path.insert(0, "/opt/trn_rl_repo")

import numpy as np
import ml_dtypes

B, T, D = 8, 256, 1024
NL, DL = 50, 512

# Per-group (dtype, n_labels). '8' = fp8 e3m4 (W scaled by W8_SCALE), 'b' = bf16.
GROUPS = [
    ("8", 1), ("8", 1), ("8", 2), ("8", 2),
    ("8", 4), ("8", 4), ("8", 4), ("8", 4), ("8", 4), ("8", 4),
    ("b", 4), ("b", 4), ("b", 4), ("b", 4), ("b", 4),
]
assert sum(sz for _, sz in GROUPS) == NL
N_GROUPS = len(GROUPS)
W8_SCALE = 128.0       # e3m4 W scale ('8' groups)
WD_SCALE = 1024.0      # e4m3 W scale ('d' groups, DoubleRow)
SELD_SCALE = 16.0      # e4m3 sel scale ('d' groups)
PREFETCH = 4  # wg groups issued ahead inside the main loop

# PE pre-warm tuning (count of dummy N=64 matmuls; the tile scheduler
# hoists them ahead of the DMA-gated real matmuls, so they must cover
# >=3.4us of continuous PE busy to release the HAM clock throttle AND
# end roughly when the first selpack half lands)
N_PREWARM = 86

# constpack layout (f32 columns)
C_BDEP = 0          # [128, 512] bdep broadcast
C_BIAS = 512        # [128, 50] label bias broadcast
C_BHEAD = 562       # [128, 4] bhead chunks (col c = bhead[c*128:(c+1)*128])
C_BHEAD16 = 566     # [128, 4] bhead chunks * SELD_SCALE (for 'd' sel8 copies)
C_TOT = 570

BF16 = ml_dtypes.bfloat16
E3M4 = ml_dtypes.float8_e3m4

LAST_RESULTS = None
_NC_CACHE = None


def _group_ranges():
    out = []
    n0 = 0
    for _, sz in GROUPS:
        out.append((n0, n0 + sz))
        n0 += sz
    return out


def _build_nc():
    import concourse.bacc as bacc
    import concourse.mybir as mybir
    import concourse.tile as tile

    bf = mybir.dt.bfloat16
    f8 = mybir.dt.float8e3
    f8d = mybir.dt.float8e4
    f32 = mybir.dt.float32
    Alu = mybir.AluOpType
    Act = mybir.ActivationFunctionType
    DR = mybir.MatmulPerfMode.DoubleRow

    def g_dt(dt8):
        return {"8": f8, "d": f8d}.get(dt8, bf)

    nc = bacc.Bacc(None)

    # --- DRAM I/O ---------------------------------------------------------
    # selpack/deppack: 4 chunks of 1536 cols; chunk q holds
    # [w slabs 2q,2q+1 (512 cols each) | xT slabs 2q,2q+1 (256 cols each)]
    selpack = nc.dram_tensor("selpack", [128, 6144], bf, kind="ExternalInput")
    deppack = nc.dram_tensor("deppack", [128, 6144], bf, kind="ExternalInput")
    constpack = nc.dram_tensor("constpack", [128, C_TOT], f32, kind="ExternalInput")
    ranges = _group_ranges()
    wg_dram = []
    for g, (dt8, sz) in enumerate(GROUPS):
        wg_dram.append(
            nc.dram_tensor(
                f"wg{g}", [128, sz * 2048], g_dt(dt8), kind="ExternalInput"
            )
        )
    out = nc.dram_tensor("out", [256, 64], f32, kind="ExternalOutput")

    def w_slab(sb, k):  # [128, 512] w slab k (Whead or Wdep rows k*128..)
        return sb[:, (k // 2) * 1536 + (k % 2) * 512 :][:, :512]

    def xT_slab(sb, k):  # [128, 256] activation^T slab k
        return sb[:, (k // 2) * 1536 + 1024 + (k % 2) * 256 :][:, :256]

    with tile.TileContext(nc) as tc:
        with (
            tc.sbuf_pool(name="cpool", bufs=1) as cpool,
            tc.sbuf_pool(name="persist", bufs=1) as pers,
            tc.sbuf_pool(name="wpool", bufs=PREFETCH + 1) as wpool,
            tc.sbuf_pool(name="spool", bufs=4) as spool,
            tc.psum_pool(name="ps", bufs=8) as ps,
        ):
            # --- input DMAs, just-in-time order ---------------------------
            sp_sb = cpool.tile([128, 6144], bf)
            dp_sb = cpool.tile([128, 6144], bf)
            cp_sb = cpool.tile([128, C_TOT], f32)
            wg_tiles = {}

            def issue_wg(g):
                dt8, sz = GROUPS[g]
                wt = wpool.tile(
                    [128, sz * 2048], g_dt(dt8), tag="wg", name=f"wg{g}"
                )
                nc.sync.dma_start(wt[:], wg_dram[g][:])
                wg_tiles[g] = wt

            # big transfers only: small dma_starts run at ~half the ring
            # bandwidth (128 row-descriptors need >=6KB payload each)
            nc.sync.dma_start(sp_sb[:, :3072], selpack[:, :3072])
            nc.sync.dma_start(sp_sb[:, 3072:], selpack[:, 3072:])
            nc.sync.dma_start(cp_sb[:], constpack[:])
            issue_wg(0)
            issue_wg(1)
            nc.sync.dma_start(dp_sb[:, :3072], deppack[:, :3072])
            nc.sync.dma_start(dp_sb[:, 3072:], deppack[:, 3072:])
            issue_wg(2)
            issue_wg(3)

            # --- PE pre-warm on a memset tile -----------------------------
            # Dummy matmuls during the startup DMA window keep the HAM
            # activity monitor busy so real matmuls start at 2.4GHz.
            pw = cpool.tile([128, 192], bf)
            nc.vector.memset(pw[:], 0.5)
            pw_ps = ps.tile([128, 512], f32, tag="ps", name="pw")

            def prewarm(n):
                for _ in range(n):
                    nc.tensor.matmul(
                        pw_ps[:, :64], lhsT=pw[:, :128], rhs=pw[:, 128:192],
                        start=True, stop=True,
                    )

            prewarm(N_PREWARM)

            # --- sel projection, transposed: selT[c] = [128 e, 256 t] -----
            psc = [
                ps.tile([128, 512], f32, tag="ps", name=f"psc{c}") for c in range(4)
            ]
            for k in range(8):
                for c in range(4):
                    nc.tensor.matmul(
                        psc[c][:, :256],
                        lhsT=w_slab(sp_sb, k)[:, c * 128 : (c + 1) * 128],
                        rhs=xT_slab(sp_sb, k),
                        start=(k == 0),
                        stop=(k == 7),
                    )
            selT = []
            for c in range(4):
                sc = pers.tile([128, 256], bf, tag=f"sel{c}", name=f"sel{c}")
                nc.scalar.activation(
                    sc[:],
                    psc[c][:, :256],
                    Act.Identity,
                    bias=cp_sb[:, C_BHEAD + c : C_BHEAD + c + 1],
                    scale=1.0,
                )
                selT.append(sc)

            # e4m3 paired-sel copies for DoubleRow groups:
            # sel8[j] cols = sub*256 + t, sub -> e-chunk 2j+sub, scaled 16x
            sel8 = []
            if any(dt8 == "d" for dt8, _ in GROUPS):
                for j in range(2):
                    s8 = pers.tile([128, 512], f8d, tag=f"sel8_{j}", name=f"sel8_{j}")
                    for sub in range(2):
                        nc.scalar.activation(
                            s8[:, sub * 256 : (sub + 1) * 256],
                            psc[2 * j + sub][:, :256],
                            Act.Identity,
                            bias=cp_sb[:, C_BHEAD16 + 2 * j + sub : C_BHEAD16 + 2 * j + sub + 1],
                            scale=SELD_SCALE,
                        )
                    sel8.append(s8)

            # --- output accumulators --------------------------------------
            out_sb = []
            for m in range(2):
                om = pers.tile([128, 64], f32, tag=f"out{m}", name=f"out{m}")
                out_sb.append(om)

            dep_label = []
            deferred = []

            DRAIN_SCALE = {
                "8": 1.0 / W8_SCALE,
                "d": 1.0 / (WD_SCALE * SELD_SCALE),
                "b": 1.0,
            }

            def drain_bank(g, n, m, pbt):
                prod = spool.tile(
                    [128, 512], f32, tag="prod", name=f"prod_{g}_{n}_{m}"
                )
                nc.vector.scalar_tensor_tensor(
                    out=prod[:],
                    in0=pbt[:],
                    scalar=DRAIN_SCALE[GROUPS[g][0]],
                    in1=dep_label[m][:],
                    op0=Alu.mult,
                    op1=Alu.mult,
                    accum_out=out_sb[m][:, n : n + 1],
                )

            def do_group(g, defer_drain=False):
                dt8, _ = GROUPS[g]
                n0, n1 = ranges[g]
                wg_sb = wg_tiles[g]
                wg_3d = wg_sb.rearrange("p (k d) -> p k d", d=512)
                for li, n in enumerate(range(n0, n1)):
                    for m in range(2):
                        pbt = ps.tile(
                            [128, 512], f32, tag="ps", name=f"pb_{g}_{li}_{m}"
                        )
                        if dt8 == "d":
                            for j in range(2):
                                s8v = sel8[j].rearrange("p (s t) -> p s t", s=2)
                                nc.tensor.matmul(
                                    pbt[:],
                                    lhsT=s8v[:, :, m * 128 : (m + 1) * 128],
                                    rhs=wg_3d[:, li * 4 + 2 * j : li * 4 + 2 * j + 2, :],
                                    start=(j == 0),
                                    stop=(j == 1),
                                    perf_mode=DR,
                                )
                        else:
                            for k in range(4):
                                nc.tensor.matmul(
                                    pbt[:],
                                    lhsT=selT[k][:, m * 128 : (m + 1) * 128],
                                    rhs=wg_sb[
                                        :, (li * 4 + k) * 512 : (li * 4 + k + 1) * 512
                                    ],
                                    start=(k == 0),
                                    stop=(k == 3),
                                )
                        if defer_drain:
                            deferred.append((g, n, m, pbt))
                        else:
                            drain_bank(g, n, m, pbt)
                if g + PREFETCH < N_GROUPS:
                    issue_wg(g + PREFETCH)

            # groups 0,1 run before dep_label exists; their drains wait.
            do_group(0, defer_drain=True)
            do_group(1, defer_drain=True)

            # --- dep projection: dep_label[m] = [128 t, 512 d] f32 --------
            pd = [ps.tile([128, 512], f32, tag="ps", name=f"pd{m}") for m in range(2)]
            for k in range(8):
                for m in range(2):
                    nc.tensor.matmul(
                        pd[m][:],
                        lhsT=xT_slab(dp_sb, k)[:, m * 128 : (m + 1) * 128],
                        rhs=w_slab(dp_sb, k),
                        start=(k == 0),
                        stop=(k == 7),
                    )
            for m in range(2):
                dl = pers.tile([128, 512], f32, tag=f"dl{m}", name=f"dl{m}")
                nc.vector.tensor_tensor(
                    dl[:], pd[m][:], cp_sb[:, C_BDEP : C_BDEP + 512], Alu.add
                )
                dep_label.append(dl)

            for (g, n, m, pbt) in deferred:
                drain_bank(g, n, m, pbt)
            deferred.clear()

            for g in range(2, N_GROUPS):
                do_group(g)

            # --- add label bias; store each half as soon as it's ready ----
            fin = pers.tile([128, 128], f32, tag="fin", name="fin")
            out_v = out.rearrange("(m p) n -> p m n", m=2)
            fin_v = fin.rearrange("p (m n) -> p m n", m=2)
            for m in range(2):
                nc.vector.tensor_tensor(
                    fin[:, m * 64 : m * 64 + NL],
                    out_sb[m][:, :NL],
                    cp_sb[:, C_BIAS : C_BIAS + NL],
                    Alu.add,
                )
                nc.sync.dma_start(
                    out_v[:, m : m + 1, :NL], fin_v[:, m : m + 1, :NL]
                )

    nc.finalize()
    return nc


def _pack_w(Wm):  # [1024, 512] -> [128, 4096] slab-major
    return Wm.reshape(8, 128, 512).transpose(1, 0, 2).reshape(128, 4096)


def _pack_x(x):  # [256, 1024] -> [128, 2048] slab-major (x^T chunks)
    return x.T.reshape(8, 128, 256).transpose(1, 0, 2).reshape(128, 2048)


def _pack_chunks(w_h, xT_h):
    """Interleave [w slab 2q, 2q+1 | xT slab 2q, 2q+1] -> [128, 6144]."""
    cols = []
    for q in range(4):
        cols.append(w_h[:, q * 1024 : (q + 1) * 1024])
        cols.append(xT_h[:, q * 512 : (q + 1) * 512])
    return np.concatenate(cols, axis=1)


def _stage_shared(Wdep, bdep, Whead, bhead, W, bias):
    whead_h = _pack_w(Whead)
    wdep_h = _pack_w(Wdep)

    # W[n, d, e] -> WT[n, k, p, d] = W[n, d, k*128+p]
    WT = np.ascontiguousarray(W.transpose(0, 2, 1)).reshape(NL, 4, 128, 512)
    wg_h = {}
    for g, (n0, n1) in enumerate(_group_ranges()):
        blk = WT[n0:n1]  # [ng, 4, 128, 512]
        ng = n1 - n0
        flat = blk.transpose(2, 0, 1, 3).reshape(128, ng * 2048)
        if GROUPS[g][0] == "8":
            wg_h[f"wg{g}"] = np.asarray(flat * W8_SCALE, dtype=E3M4)
        elif GROUPS[g][0] == "d":
            wg_h[f"wg{g}"] = np.asarray(
                np.clip(flat * WD_SCALE, -240.0, 240.0),
                dtype=ml_dtypes.float8_e4m3,
            )
        else:
            wg_h[f"wg{g}"] = np.asarray(flat, dtype=BF16)

    constpack = np.zeros((128, C_TOT), dtype=np.float32)
    constpack[:, C_BDEP : C_BDEP + 512] = bdep[None, :]
    constpack[:, C_BIAS : C_BIAS + NL] = bias[None, :]
    constpack[:, C_BHEAD : C_BHEAD + 4] = bhead.reshape(4, 128).T
    constpack[:, C_BHEAD16 : C_BHEAD16 + 4] = SELD_SCALE * bhead.reshape(4, 128).T

    return {
        "whead_h": whead_h,
        "wdep_h": wdep_h,
        "wg_h": wg_h,
        "constpack": constpack,
    }


def _stage_core(shared, dep_b, head_b, idx_b):
    head_sel = head_b[np.asarray(idx_b, dtype=np.int64)]  # host gather [256,1024]
    selpack = np.ascontiguousarray(
        _pack_chunks(shared["whead_h"], _pack_x(head_sel))
    ).astype(BF16)
    deppack = np.ascontiguousarray(
        _pack_chunks(shared["wdep_h"], _pack_x(dep_b))
    ).astype(BF16)
    m = {"selpack": selpack, "deppack": deppack, "constpack": shared["constpack"]}
    m.update(shared["wg_h"])
    return m


def kernel(dep, head, head_indices, mask, Wdep, bdep, Whead, bhead, W, bias):
    global LAST_RESULTS, _NC_CACHE
    from concourse.bass_utils import run_bass_kernel_spmd

    dep = np.asarray(dep, dtype=np.float32)
    head = np.asarray(head, dtype=np.float32)
    head_indices = np.asarray(head_indices)
    Wdep = np.asarray(Wdep, dtype=np.float32)
    bdep = np.asarray(bdep, dtype=np.float32)
    Whead = np.asarray(Whead, dtype=np.float32)
    bhead = np.asarray(bhead, dtype=np.float32)
    W = np.asarray(W, dtype=np.float32)
    bias = np.asarray(bias, dtype=np.float32)

    if _NC_CACHE is None:
        _NC_CACHE = _build_nc()
    nc = _NC_CACHE

    shared = _stage_shared(Wdep, bdep, Whead, bhead, W, bias)
    in_maps = [
        _stage_core(shared, dep[b], head[b], head_indices[b]) for b in range(B)
    ]

    res = run_bass_kernel_spmd(nc, in_maps, list(range(B)))
    LAST_RESULTS = res
    outs = [
        np.asarray(res.results[b]["out"][:, :NL], dtype=np.float32) for b in range(B)
    ]
    return np.stack(outs, axis=0)
